# revision 3
# baseline (speedup 1.0000x reference)
"""Dynamic per-pixel 3x3 filtering on 8 Trainium2 NeuronCores.

out[b,c,y,x] = sum_{ki,kj} img[b,c,y+ki-1,x+kj-1] * kernels[b,c,ki*3+kj,y,x]
(zero padding outside the image).

Sharding: pure data parallel, one batch sample per core (B=8, 8 cores).

Per-core layout: each channel's [512, 512] image plane is viewed as
[128 partitions, 4 blocks, 512 cols] (row r = block*128 + partition).
Row-shifted variants (y-1 / y+1) are materialized with SBUF->SBUF DMA
(partition-shift is impossible on the lockstep compute engines but free
on the DMA AXI ports, which don't contend with HBM-side engine traffic).
Column shifts are plain free-dim AP offsets.

The 9 multiply + 8 accumulate elementwise passes per channel are split
between the Vector engine (6 taps) and GPSIMD (3 taps + final merge):
f32 tensor_tensor runs in 1x mode on DVE, which never touches the
DVE/GPSIMD shared SBUF port, so both engines stream concurrently.
All DMA goes through HWDGE (nc.sync) so GPSIMD is never needed for
descriptor generation.
"""

from contextlib import ExitStack

import numpy as np

import concourse.bacc as bacc
import concourse.mybir as mybir
import concourse.tile as tile
from concourse.bass_utils import run_bass_kernel_spmd

C, H, W = 3, 512, 512
KK = 9
NCORES = 8
P = 128
NB = H // P          # 4 row blocks per channel
FW = NB * W          # 2048 free-dim width of a channel mega-tile
F32 = mybir.dt.float32

# Taps: t = ki*3 + kj; row shift = ki-1 (top/mid/bot), col shift = kj-1.
DVE_TAPS = [4, 3, 5, 7, 6, 8]   # first tap must be a dx=0 (full-width) tap
GPS_TAPS = [1, 0, 2]


def _r3(ap):
    """[128, FW] -> [128, NB, W] block view of a channel mega-tile."""
    return ap.rearrange("p (b x) -> p b x", x=W)


def _emit(nc, tc, ctx):
    img = nc.dram_tensor("img", (C, H, W), F32, kind="ExternalInput").ap()
    ker = nc.dram_tensor("kernels", (C, KK, H, W), F32, kind="ExternalInput").ap()
    out = nc.dram_tensor("out", (C, H, W), F32, kind="ExternalOutput").ap()

    v_pool = ctx.enter_context(tc.tile_pool(name="v", bufs=2))
    z_pool = ctx.enter_context(tc.tile_pool(name="z", bufs=1))
    k_pool = ctx.enter_context(tc.tile_pool(name="k", bufs=12))
    acc_pool = ctx.enter_context(tc.tile_pool(name="acc", bufs=2))
    tmp_pool = ctx.enter_context(tc.tile_pool(name="tmp", bufs=1))

    zrow = z_pool.tile([P, W], F32, tag="zrow")
    nc.gpsimd.memset(zrow[:, :], 0.0)

    for c in range(C):
        # img rows for this channel: mid[p, b*W + x] = img[c, b*128 + p, x]
        mid = v_pool.tile([P, FW], F32, tag="mid")
        nc.sync.dma_start(
            _r3(mid[:, :]), img[c].rearrange("(b p) x -> p b x", p=P)
        )
        # top[p, b, x] = img row (b*128 + p - 1), zeros above the image
        top = v_pool.tile([P, FW], F32, tag="top")
        nc.sync.dma_start(top[1:P, :], mid[0 : P - 1, :])
        nc.sync.dma_start(top[0:1, W:FW], mid[P - 1 : P, 0 : FW - W])
        nc.sync.dma_start(top[0:1, 0:W], zrow[0:1, :])
        # bot[p, b, x] = img row (b*128 + p + 1), zeros below the image
        bot = v_pool.tile([P, FW], F32, tag="bot")
        nc.sync.dma_start(bot[0 : P - 1, :], mid[1:P, :])
        nc.sync.dma_start(bot[P - 1 : P, 0 : FW - W], mid[0:1, W:FW])
        nc.sync.dma_start(bot[P - 1 : P, FW - W : FW], zrow[P - 1 : P, :])

        kts = []
        for t in range(KK):
            kt = k_pool.tile([P, FW], F32, tag="kt")
            nc.sync.dma_start(
                _r3(kt[:, :]), ker[c, t].rearrange("(b p) x -> p b x", p=P)
            )
            kts.append(kt)

        acc = acc_pool.tile([P, FW], F32, tag="acc")    # DVE accumulator
        accg = acc_pool.tile([P, FW], F32, tag="accg")  # GPSIMD accumulator
        tmp = tmp_pool.tile([P, FW], F32, tag="tmp")
        tmpg = tmp_pool.tile([P, FW], F32, tag="tmpg")

        vs = [top, mid, bot]

        def do_taps(eng, a, tm, taps):
            first = True
            for t in taps:
                ki, kj = divmod(t, 3)
                v, dx = vs[ki], kj - 1
                if dx == 0:
                    if first:
                        eng.tensor_mul(a[:, :], v[:, :], kts[t][:, :])
                    else:
                        eng.tensor_mul(tm[:, :], v[:, :], kts[t][:, :])
                        eng.tensor_add(a[:, :], a[:, :], tm[:, :])
                else:
                    a3, v3, k3 = _r3(a[:, :]), _r3(v[:, :]), _r3(kts[t][:, :])
                    tsl = _r3(tm[:, :])[:, :, 0 : W - 1]
                    if dx < 0:
                        asl, vsl, ksl = a3[:, :, 1:W], v3[:, :, 0 : W - 1], k3[:, :, 1:W]
                    else:
                        asl, vsl, ksl = a3[:, :, 0 : W - 1], v3[:, :, 1:W], k3[:, :, 0 : W - 1]
                    eng.tensor_mul(tsl, vsl, ksl)
                    eng.tensor_add(asl, asl, tsl)
                first = False

        do_taps(nc.vector, acc, tmp, DVE_TAPS)
        do_taps(nc.gpsimd, accg, tmpg, GPS_TAPS)
        nc.gpsimd.tensor_add(accg[:, :], accg[:, :], acc[:, :])

        nc.sync.dma_start(
            out[c].rearrange("(b p) x -> p b x", p=P), _r3(accg[:, :])
        )


_NC_CACHE = []


def _build():
    nc = bacc.Bacc(
        "TRN2",
        target_bir_lowering=False,
        debug=False,
        enable_asserts=True,
        num_devices=1,
    )
    with tile.TileContext(nc) as tc:
        with ExitStack() as ctx:
            _emit(nc, tc, ctx)
    nc.compile()
    return nc


def kernel(img, kernels):
    """img: [8, 3, 512, 512] f32; kernels: [8, 3, 9, 512, 512] f32.
    Returns [8, 3, 512, 512] f32."""
    if not _NC_CACHE:
        _NC_CACHE.append(_build())
    nc = _NC_CACHE[0]
    img = np.asarray(img, dtype=np.float32)
    kernels = np.asarray(kernels, dtype=np.float32)
    in_maps = [
        {
            "img": np.ascontiguousarray(img[b]),
            "kernels": np.ascontiguousarray(kernels[b]),
        }
        for b in range(NCORES)
    ]
    res = run_bass_kernel_spmd(nc, in_maps, core_ids=list(range(NCORES)))
    return np.stack([res.results[b]["out"] for b in range(NCORES)], axis=0)
